# revision 2
# baseline (speedup 1.0000x reference)
"""Balanced-span variable-split all-to-all (MoE dispatch) for 8 trn2 cores.

The global valid output space (all ranks' received rows, concatenated in
(rank, row) order) is cut into 8 equal-row pieces; core k produces piece k
into its own [M, H] buffer at piece-local offsets that preserve the final
(rank, row) order. Fragments (chunk-within-piece intersections, contiguous
src/dst row ranges) are emitted as STATIC dma_starts inside an 8-way
Switch on partition id -- each core runs only its own straight-line body.
No offset tables, no values_load, no bounds checks, no skip slots: every
fragment is one large DMA whose descriptors spray evenly over all 16 SDMA
engines (compile-time AP balancing). Fragments are split between the two
HWDGE queues (sync/scalar) balanced by bytes. The compiled program is
cached per fragment-structure signature. Host unshard copies each piece's
rank-spans into the final [W, M, H] zeros buffer.
"""
import os
import sys
import types

import numpy as np

W, M, H = 8, 16384, 1024

_cache = {}


def _install_profshim():
    if "antenv.axon_hooks" in sys.modules:
        return
    try:
        from trn_agent_boot.trn_boot import _ntff_profile_via_ctypes
        hook = _ntff_profile_via_ctypes("/opt/axon/libaxon_pjrt.so")
    except Exception:
        hook = None
    mod = types.ModuleType("antenv.axon_hooks")
    mod.get_axon_ntff_profile_hook = lambda: hook
    mod.set_axon_ntff_profile_hook = lambda h: None
    sys.modules["antenv.axon_hooks"] = mod


def _plan_pieces(splits):
    """Cut the concatenated valid space into 8 pieces; return per-piece
    fragment lists [(src_row, dst_local_row, n)] and the per-piece
    final-output span map [(r, row_start, row_end, local_start)]."""
    sp = splits.astype(np.int64)
    in_off = sp.cumsum(1) - sp          # [s, r]
    recv = sp.T                          # [r, s]
    out_off = recv.cumsum(1) - recv      # [r, s]
    totals = recv.sum(1)                 # [r]
    tot_prefix = np.concatenate([[0], totals.cumsum()])
    G = int(tot_prefix[-1])

    cuts = [round(k * G / W) for k in range(W + 1)]

    # global chunk list in (r, s) order with global start positions
    chunks = []  # (g_start, n, src_row)
    for r in range(W):
        for s in range(W):
            n = int(sp[s, r])
            if n == 0:
                continue
            g = int(tot_prefix[r] + out_off[r, s])
            chunks.append((g, n, s * M + int(in_off[s, r])))

    frags = [[] for _ in range(W)]
    spans = [[] for _ in range(W)]
    for k in range(W):
        a, b = cuts[k], cuts[k + 1]
        if a == b:
            continue
        for g, n, src in chunks:
            lo, hi = max(g, a), min(g + n, b)
            if lo >= hi:
                continue
            frags[k].append((src + (lo - g), lo - a, hi - lo))
        # final-output spans covered by this piece
        for r in range(W):
            ra, rb = int(tot_prefix[r]), int(tot_prefix[r + 1])
            lo, hi = max(ra, a), min(rb, b)
            if lo >= hi:
                continue
            spans[k].append((r, lo - ra, hi - ra, lo - a))
    return frags, spans


def _queue_split(frag_list):
    """Split one core's fragments across the 2 HWDGE queues, balancing
    bytes. Within each queue, descending size (small tail last)."""
    order = sorted(frag_list, key=lambda f: -f[2])
    qa, qb = [], []
    na = nb = 0
    for f in order:
        if na <= nb:
            qa.append(f)
            na += f[2]
        else:
            qb.append(f)
            nb += f[2]
    return qa, qb


def _build_kernel(per_core_queues):
    import concourse.bacc as bacc
    import concourse.mybir as mybir

    F32 = mybir.dt.float32

    nc = bacc.Bacc("TRN2", target_bir_lowering=False, debug=False, num_devices=W)
    inp = nc.dram_tensor("inp", [W * M, H], F32, kind="ExternalInput")
    out = nc.dram_tensor("out", [M, H], F32, kind="ExternalOutput")

    sp, act = nc.sync, nc.scalar
    sem_sp = nc.alloc_semaphore("sem_sp")
    sem_act = nc.alloc_semaphore("sem_act")
    sp.sem_clear(sem_sp)
    act.sem_clear(sem_act)
    pid_sp = sp.partition_id()
    pid_act = act.partition_id()

    for k in nc.Switch(engines=[sp, act], index=[pid_sp, pid_act], n=W):
        qa, qb = per_core_queues[k]
        for src, dst, n in qa:
            sp.dma_start(out=out[dst:dst + n, :],
                         in_=inp[src:src + n, :]).then_inc(sem_sp, 16)
        for src, dst, n in qb:
            act.dma_start(out=out[dst:dst + n, :],
                          in_=inp[src:src + n, :]).then_inc(sem_act, 16)
        if qa:
            sp.wait_ge(sem_sp, 16 * len(qa))
        if qb:
            act.wait_ge(sem_act, 16 * len(qb))
    nc.compile()
    return nc


last_exec_time_ns = None


def kernel(input, splits, num_sm=None, **_unused):
    global last_exec_time_ns
    _install_profshim()
    from concourse.bass_utils import run_bass_kernel_spmd

    input = np.asarray(input, dtype=np.float32)
    splits = np.asarray(splits, dtype=np.int32)
    assert input.shape == (W, M, H), input.shape
    assert splits.shape == (W, W), splits.shape

    frags, spans = _plan_pieces(splits)
    if not any(frags):
        last_exec_time_ns = 0
        return np.zeros((W, M, H), dtype=np.float32)

    per_core_queues = [_queue_split(f) for f in frags]
    key = tuple(
        (tuple(qa), tuple(qb)) for qa, qb in per_core_queues
    )
    if key not in _cache:
        _cache[key] = _build_kernel(per_core_queues)
    nc = _cache[key]

    flat = np.ascontiguousarray(input.reshape(W * M, H))
    in_maps = [{"inp": flat} for _ in range(W)]

    trace = bool(int(os.environ.get("A2A_PROFILE", "0")))
    res = run_bass_kernel_spmd(
        nc, in_maps, core_ids=list(range(W)),
        trace=trace, trace_cores=list(range(W)) if trace else None,
    )
    last_exec_time_ns = res.exec_time_ns

    out = np.zeros((W, M, H), dtype=np.float32)
    for k in range(W):
        buf = res.results[k]["out"]
        for r, ra, rb, la in spans[k]:
            out[r, ra:rb] = buf[la:la + (rb - ra)]
    return out


# revision 3
# speedup vs baseline: 1.0247x; 1.0247x over previous
"""Balanced-span variable-split all-to-all (MoE dispatch) for 8 trn2 cores.

The global valid output space (all ranks' received rows, concatenated in
(rank, row) order) is cut into 8 equal-row pieces; core k produces piece k
into its own [M, H] buffer at piece-local offsets that preserve the final
(rank, row) order. Fragments (chunk-within-piece intersections, contiguous
src/dst row ranges) are emitted as STATIC dma_starts inside an 8-way
Switch on partition id -- each core runs only its own straight-line body.
No offset tables, no values_load, no bounds checks, no skip slots: every
fragment is one large DMA whose descriptors spray evenly over all 16 SDMA
engines (compile-time AP balancing). Fragments are split between the two
HWDGE queues (sync/scalar) balanced by bytes. The compiled program is
cached per fragment-structure signature. Host unshard copies each piece's
rank-spans into the final [W, M, H] zeros buffer.
"""
import os
import sys
import types

import numpy as np

W, M, H = 8, 16384, 1024

_cache = {}


def _install_profshim():
    if "antenv.axon_hooks" in sys.modules:
        return
    try:
        from trn_agent_boot.trn_boot import _ntff_profile_via_ctypes
        hook = _ntff_profile_via_ctypes("/opt/axon/libaxon_pjrt.so")
    except Exception:
        hook = None
    mod = types.ModuleType("antenv.axon_hooks")
    mod.get_axon_ntff_profile_hook = lambda: hook
    mod.set_axon_ntff_profile_hook = lambda h: None
    sys.modules["antenv.axon_hooks"] = mod


def _plan_pieces(splits):
    """Cut the concatenated valid space into 8 pieces; return per-piece
    fragment lists [(src_row, dst_local_row, n)] and the per-piece
    final-output span map [(r, row_start, row_end, local_start)]."""
    sp = splits.astype(np.int64)
    in_off = sp.cumsum(1) - sp          # [s, r]
    recv = sp.T                          # [r, s]
    out_off = recv.cumsum(1) - recv      # [r, s]
    totals = recv.sum(1)                 # [r]
    tot_prefix = np.concatenate([[0], totals.cumsum()])
    G = int(tot_prefix[-1])

    cuts = [round(k * G / W) for k in range(W + 1)]

    # global chunk list in (r, s) order with global start positions
    chunks = []  # (g_start, n, src_row)
    for r in range(W):
        for s in range(W):
            n = int(sp[s, r])
            if n == 0:
                continue
            g = int(tot_prefix[r] + out_off[r, s])
            chunks.append((g, n, s * M + int(in_off[s, r])))

    frags = [[] for _ in range(W)]
    spans = [[] for _ in range(W)]
    for k in range(W):
        a, b = cuts[k], cuts[k + 1]
        if a == b:
            continue
        for g, n, src in chunks:
            lo, hi = max(g, a), min(g + n, b)
            if lo >= hi:
                continue
            frags[k].append((src + (lo - g), lo - a, hi - lo))
        # final-output spans covered by this piece
        for r in range(W):
            ra, rb = int(tot_prefix[r]), int(tot_prefix[r + 1])
            lo, hi = max(ra, a), min(rb, b)
            if lo >= hi:
                continue
            spans[k].append((r, lo - ra, hi - ra, lo - a))
    return frags, spans


CHUNK = 256  # rows per dma_start (1 MiB): 16 descriptors, one per SDMA engine


def _queue_split(frag_list):
    """Chunk fragments to <=CHUNK rows (16 x 64KB descriptors -> even
    engine spray, no HWDGE ring backpressure), split across the 2 HWDGE
    queues balanced by bytes, sub-chunk remainders issued last so every
    engine's tail is short."""
    full, rest = [], []
    for src, dst, n in frag_list:
        o = 0
        while n - o >= CHUNK:
            full.append((src + o, dst + o, CHUNK))
            o += CHUNK
        if n - o:
            rest.append((src + o, dst + o, n - o))
    rest.sort(key=lambda f: -f[2])
    qa, qb = [], []
    na = nb = 0
    for f in full + rest:
        if na <= nb:
            qa.append(f)
            na += f[2]
        else:
            qb.append(f)
            nb += f[2]
    return qa, qb


def _build_kernel(per_core_queues):
    import concourse.bacc as bacc
    import concourse.mybir as mybir

    F32 = mybir.dt.float32

    nc = bacc.Bacc("TRN2", target_bir_lowering=False, debug=False, num_devices=W)
    inp = nc.dram_tensor("inp", [W * M, H], F32, kind="ExternalInput")
    out = nc.dram_tensor("out", [M, H], F32, kind="ExternalOutput")

    sp, act = nc.sync, nc.scalar
    sem_sp = nc.alloc_semaphore("sem_sp")
    sem_act = nc.alloc_semaphore("sem_act")
    sp.sem_clear(sem_sp)
    act.sem_clear(sem_act)
    pid_sp = sp.partition_id()
    pid_act = act.partition_id()

    for k in nc.Switch(engines=[sp, act], index=[pid_sp, pid_act], n=W):
        qa, qb = per_core_queues[k]
        for src, dst, n in qa:
            sp.dma_start(out=out[dst:dst + n, :],
                         in_=inp[src:src + n, :]).then_inc(sem_sp, 16)
        for src, dst, n in qb:
            act.dma_start(out=out[dst:dst + n, :],
                          in_=inp[src:src + n, :]).then_inc(sem_act, 16)
        if qa:
            sp.wait_ge(sem_sp, 16 * len(qa))
        if qb:
            act.wait_ge(sem_act, 16 * len(qb))
    nc.compile()
    return nc


last_exec_time_ns = None


def kernel(input, splits, num_sm=None, **_unused):
    global last_exec_time_ns
    _install_profshim()
    from concourse.bass_utils import run_bass_kernel_spmd

    input = np.asarray(input, dtype=np.float32)
    splits = np.asarray(splits, dtype=np.int32)
    assert input.shape == (W, M, H), input.shape
    assert splits.shape == (W, W), splits.shape

    frags, spans = _plan_pieces(splits)
    if not any(frags):
        last_exec_time_ns = 0
        return np.zeros((W, M, H), dtype=np.float32)

    per_core_queues = [_queue_split(f) for f in frags]
    key = tuple(
        (tuple(qa), tuple(qb)) for qa, qb in per_core_queues
    )
    if key not in _cache:
        _cache[key] = _build_kernel(per_core_queues)
    nc = _cache[key]

    flat = np.ascontiguousarray(input.reshape(W * M, H))
    in_maps = [{"inp": flat} for _ in range(W)]

    trace = bool(int(os.environ.get("A2A_PROFILE", "0")))
    res = run_bass_kernel_spmd(
        nc, in_maps, core_ids=list(range(W)),
        trace=trace, trace_cores=list(range(W)) if trace else None,
    )
    last_exec_time_ns = res.exec_time_ns

    out = np.zeros((W, M, H), dtype=np.float32)
    for k in range(W):
        buf = res.results[k]["out"]
        for r, ra, rb, la in spans[k]:
            out[r, ra:rb] = buf[la:la + (rb - ra)]
    return out
